# revision 132
# baseline (speedup 1.0000x reference)
"""Grouped-Query Attention (B=2, T=2048, E=2048, 16 Q heads / 4 KV heads, RoPE,
causal) as a Bass/Tile kernel on 8 Trainium2 NeuronCores.

Sharding: core c = 4*b + h handles batch b (of 2) and KV-head group h (of 4,
i.e. 4 q-heads + 1 kv head).  Each core computes its QKV projections (channel
sharded), RoPE, causal attention for its 4 q heads, and a partial
out-projection over its 512 channels of Wo.  The 4 partial out-proj results per
batch are summed on the host during unsharding (row-sharded out_proj).

The PE (tensor engine) is the bottleneck, so everything that is not a real
GEMM is off-loaded: softmax denominators come from a DVE elementwise
accumulation of the exp tiles finished by a gpsimd partition_all_reduce
(which yields the colsum already broadcast, so normalize = reduce + recip +
mul entirely off-PE), and RoPE rotation is two partition-offset DVE copies
against a sign-folded sin table (no PE rotation matmuls).  All matmul
operands are bf16 (fp32 PSUM accumulation); scores use ragged causal widths
with score tiles emitted two-plus steps ahead across head boundaries;
projection of head h+1 is interleaved into attention of head h; and the
out-projection's first chunks ride inside the last head's attention (which
is otherwise exp-bound) so only a short pass B remains at the end.
"""

import os
import sys

import numpy as np

try:
    import concourse.bass as bass
except ModuleNotFoundError:  # fresh grading dir: point at the in-container repo
    for p in ("/opt/trn_rl_repo", "/root/.axon_site/_ro/trn_rl_repo"):
        if os.path.isdir(p) and p not in sys.path:
            sys.path.insert(0, p)
    import concourse.bass as bass

from contextlib import ExitStack

from ml_dtypes import bfloat16

import concourse.bass_isa as bass_isa
import concourse.tile as tile
from concourse import bacc, mybir
from concourse.bass_utils import run_bass_kernel_spmd

# ---- problem constants (hardcoded per contract) ----
B, T, E = 2, 2048, 2048
N_QHEAD, N_KVHEAD = 16, 4
GROUP = N_QHEAD // N_KVHEAD          # 4 q heads per kv head
D = E // N_QHEAD                     # 128 head dim
KV = E // GROUP                      # 512 kv dim
ROPE_BASE = 10000.0
N_CORES = 8

P = 128                              # partitions
H = P // 2                           # rope half
ET = E // P                          # 16 e-tiles
TT = T // P                          # 16 token tiles
TC = 512                             # q-chunk / PSUM-bank width (fp32)
NTC = T // TC                        # 4 chunks

F32 = mybir.dt.float32
F32R = mybir.dt.float32r
BF16 = mybir.dt.bfloat16

_CACHE = {}


def _build_program():
    """Build + compile the (SPMD-identical) Bass program once per process."""
    if "nc" in _CACHE:
        return _CACHE["nc"]

    nc = bacc.Bacc("TRN2", target_bir_lowering=False, debug=False,
                   num_devices=N_CORES)

    dram = {}
    def din(name, shape, dt=BF16):
        dram[name] = nc.dram_tensor(name, list(shape), dt,
                                    kind="ExternalInput").ap()
    din("xt", (NTC, P, ET, TC))         # x[b].T: (tq-chunk, p, e-tile, t)
                                        # partition-major: contiguous reads
    din("wq", (P, GROUP * ET * P))      # WqT tiles, (ct, e) major
    din("wk", (P, ET * P))
    din("wv", (P, ET * P))
    din("wo", (P, GROUP * TT * P))      # WoS.T tiles (ct, jt) major
    din("bias6", (P, 6), F32)           # per-ctile biases: 4x bq, k, v
    din("bo16", (P, TT), F32)           # bo per j-tile (zeros on h!=0 cores)
    din("cos", (P, T))                  # rope tables, shared q/k (the 1/
    din("sin", (P, T))                  # sqrt(D) q-scale is folded into Wq);
                                        # sin sign-folded: rows<64 negated
    din("ident", (P, P))                # identity (for PE transpose)
    din("maskd", (P, P))                # diagonal-block causal mask (tq>=tk)
    outt = nc.dram_tensor("outt", [TT, P, T], BF16, kind="ExternalOutput").ap()

    with tile.TileContext(nc) as tc:
        with ExitStack() as ctx, nc.allow_low_precision(
                reason="bf16 matmul operands; accumulation stays fp32 in PSUM"):
            persist = ctx.enter_context(tc.tile_pool(name="persist", bufs=1))

            def ptile(shape, name, dt=BF16):
                return persist.tile(shape, dt, tag=name, name=name)

            # ---------- persistent SBUF tiles ----------
            wq_sb = ptile([P, GROUP * ET * P], "wq_sb")
            wk_sb = ptile([P, ET * P], "wk_sb")
            wv_sb = ptile([P, ET * P], "wv_sb")
            wo_sb = ptile([P, GROUP * TT * P], "wo_sb")
            bias6_sb = ptile([P, 8], "bias6_sb", F32)
            bo16_sb = ptile([P, TT], "bo16_sb", F32)
            cos_sb = ptile([P, T], "cos_sb")
            sin_sb = ptile([P, T], "sin_sb")
            ident_sb = ptile([P, P], "ident_sb")
            maskd_sb = ptile([P, P], "maskd_sb")
            x_sb = ptile([P, NTC * ET * TC], "x_sb")     # full xT, resident
            kT_sb = ptile([P, T], "kT_sb")
            vT_sb = ptile([P, T], "vT_sb")
            vtok_sb = ptile([P, T], "vtok_sb")
            qA_sb = ptile([P, T], "qA_sb")               # qT, heads alternate
            qB_sb = ptile([P, T], "qB_sb")
            y_sb = ptile([P, GROUP * T], "y_sb")

            # SBUF pools
            egp = ctx.enter_context(tc.tile_pool(name="egp", bufs=7))
            eap = ctx.enter_context(tc.tile_pool(name="eap", bufs=1))
            osb = ctx.enter_context(tc.tile_pool(name="osb", bufs=8))
            nsb = ctx.enter_context(tc.tile_pool(name="nsb", bufs=2))
            # PSUM pools: 4 + 2 + 2 = 8 banks
            pyps = ctx.enter_context(tc.tile_pool(name="pyps", bufs=4, space="PSUM"))
            pscr = ctx.enter_context(tc.tile_pool(name="pscr", bufs=2, space="PSUM"))
            pmx = ctx.enter_context(tc.tile_pool(name="pmx", bufs=2, space="PSUM"))

            # ---------- load constants (ordered by first use) ----------
            # x stream rides the SP HWDGE queue; weights/tables ride the Act
            # HWDGE queue so the first matmul is gated only by wk + x split 0.
            x3 = x_sb[:].rearrange("p (c e t) -> p c e t", c=NTC, e=ET)

            def dma_x(xc, nsplit=4, eng=None):
                xd = dram["xt"][xc]
                step = ET // nsplit
                for q in range(nsplit):
                    (eng or nc.sync).dma_start(
                        x3[:, xc, q * step:(q + 1) * step, :],
                        xd[:, q * step:(q + 1) * step, :])

            def dma_x_splits(xc, bounds, eng=None):
                xd = dram["xt"][xc]
                for a, b in zip(bounds[:-1], bounds[1:]):
                    (eng or nc.sync).dma_start(x3[:, xc, a:b, :],
                                               xd[:, a:b, :])

            # priming: tiny first pieces so the first k-proj matmul starts
            # as early as possible, then progressively larger
            # critical stream on the SP queue in exact need-order; the Act
            # queue carries only the non-gating loads (rope tables, later
            # heads' weights) so the engines mostly see the critical order
            # the two HWDGE queues generate descriptors in parallel, so
            # the startup pieces alternate: weights on SP, x on Act
            nc.sync.dma_start(wk_sb[:, 0:2 * P], dram["wk"][:, 0:2 * P])
            dma_x_splits(0, [0, 2], eng=nc.scalar)
            nc.sync.dma_start(wk_sb[:, 2 * P:8 * P], dram["wk"][:, 2 * P:8 * P])
            dma_x_splits(0, [2, 4], eng=nc.scalar)
            nc.sync.dma_start(wk_sb[:, 8 * P:], dram["wk"][:, 8 * P:])
            nc.sync.dma_start(bias6_sb[:, 0:6], dram["bias6"][:])
            dma_x_splits(0, [4, 8], eng=nc.scalar)
            nc.sync.dma_start(wv_sb[:], dram["wv"][:])
            dma_x_splits(0, [8, 12], eng=nc.scalar)
            dma_x_splits(0, [12, 16])
            dma_x_splits(1, [0, 8, 16])
            dma_x_splits(2, [0, 8, 16])
            nc.sync.dma_start(wq_sb[:, 0:8 * P], dram["wq"][:, 0:8 * P])
            nc.sync.dma_start(wq_sb[:, 8 * P:ET * P],
                              dram["wq"][:, 8 * P:ET * P])
            dma_x_splits(3, [0, 8, 16])
            # rope tables load last: only needed by the first scores (~40us),
            # and on the Act queue they would steal engine slots from the
            # critical x1/x2 stream
            nc.sync.dma_start(cos_sb[:], dram["cos"][:])
            nc.sync.dma_start(sin_sb[:], dram["sin"][:])
            for nm, t in [("ident", ident_sb), ("maskd", maskd_sb)]:
                nc.scalar.dma_start(t[:], dram[nm][:])

            def wq_tile(ct, e):
                return wq_sb[:, (ct * ET + e) * P:(ct * ET + e + 1) * P]

            def proj_mms(pool, tag, lhs_of_e, xc, e0, e1, psum_holder):
                """Emit e-mms [e0,e1) accumulating over e into psum_holder."""
                if e0 == 0:
                    psum_holder[0] = pool.tile([P, TC], F32, tag=tag,
                                               name="pacc")
                pp = psum_holder[0]
                for e in range(e0, e1):
                    nc.tensor.matmul(pp[:], lhs_of_e(e),
                                     x_sb[:, (xc * ET + e) * TC:
                                          (xc * ET + e + 1) * TC],
                                     start=(e == 0), stop=(e == ET - 1))

            def proj_evac(dst, psum_holder, bias_col, eng="act"):
                if eng == "act":
                    nc.scalar.activation(dst, psum_holder[0][:],
                                         mybir.ActivationFunctionType.Identity,
                                         bias=bias6_sb[:, bias_col:bias_col + 1])
                else:
                    nc.vector.tensor_scalar_add(
                        dst, psum_holder[0][:],
                        bias6_sb[:, bias_col:bias_col + 1])

            # ---------- rope: partition-rotate on DVE, sign-folded sin ----
            # rot[p<64] = -u[p+64], rot[p>=64] = u[p-64]; the sign lives in
            # the sin table.  The rotate is two single-input partition-offset
            # copies (walrus only requires equal base partitions when BOTH
            # inputs are in SBUF), so rope never touches PE or PSUM.
            def rope_chunk(dst_full, cos_sb, sin_sb, c, rot_eng=None):
                cs = slice(c * TC, (c + 1) * TC)
                tmp = nsb.tile([P, TC], BF16, tag="rt", name="tmp")
                rot = rot_eng or nc.vector
                rot.tensor_copy(tmp[0:H, :], dst_full[H:P, cs])
                rot.tensor_copy(tmp[H:P, :], dst_full[0:H, cs])
                nc.vector.tensor_mul(tmp[:], tmp[:], sin_sb[:, cs])
                nc.vector.tensor_mul(dst_full[:, cs], dst_full[:, cs],
                                     cos_sb[:, cs])
                nc.vector.tensor_add(dst_full[:, cs], dst_full[:, cs],
                                     tmp[:])

            # ---------- shared scores machinery ----------
            # one eg tile per (head, k-tile); score matmuls in 512-wide
            # chunks, each followed by its exp, so the exp pipeline stays
            # fine-grained.  Tiles are emitted up to two steps ahead in
            # global (head, j) order, across head boundaries.
            egs = {}

            def qbuf(hh):
                return qA_sb if hh % 2 == 0 else qB_sb

            TILES = [(hh, j) for hh in range(GROUP) for j in range(TT)]
            NEXT = {t: TILES[i + 1] for i, t in enumerate(TILES[:-1])}

            def score_parts(key):
                hh, j = key
                base = j * P
                w = T - base
                eg = egp.tile([P, T], BF16, tag="eg", name="eg")
                egs[key] = eg
                qq = qbuf(hh)
                # the last head has no projection interleave, so pmx's two
                # banks are free: alternating pools per tile doubles the
                # score-tile rotation depth exactly where exp drains slowest
                pool, tg = ((pmx, "mi") if hh == GROUP - 1 and j % 2
                            else (pscr, "sc"))

                def chunk(c0, cw):
                    def emit():
                        sps = pool.tile([P, TC], F32, tag=tg, name="sps")
                        nc.tensor.matmul(
                            sps[:, 0:cw], kT_sb[:, j * P:(j + 1) * P],
                            qq[:, base + c0:base + c0 + cw],
                            start=True, stop=True)
                        nc.scalar.activation(
                            eg[:, c0:c0 + cw], sps[:, 0:cw],
                            mybir.ActivationFunctionType.Exp)
                    return emit

                return [chunk(c0, min(TC, w - c0))
                        for c0 in range(0, w, TC)]

            # ---------- phase 1a: k + v per chunk (startup burst is only
            # wk + wv + the x stream) ----------
            hold = [None]
            for xc in range(NTC):
                cs = slice(xc * TC, (xc + 1) * TC)
                proj_mms(pyps, "acc", lambda e: wk_sb[:, e * P:(e + 1) * P],
                         xc, 0, ET, hold)
                proj_evac(kT_sb[:, cs], hold, 4)
                rope_chunk(kT_sb, cos_sb, sin_sb, xc)
                proj_mms(pyps, "acc", lambda e: wv_sb[:, e * P:(e + 1) * P],
                         xc, 0, ET, hold)
                proj_evac(vT_sb[:, cs], hold, 5)
                # v -> token-major via PE transpose; DVE evacuates so the
                # Act queue stays clear for the projection evacs
                for jj in range(xc * NTC, (xc + 1) * NTC):
                    vps = pmx.tile([P, P], BF16, tag="mi", name="vps")
                    nc.tensor.transpose(vps[:], vT_sb[:, jj * P:(jj + 1) * P],
                                        ident_sb[:])
                    nc.vector.tensor_copy(vtok_sb[:, jj * P:(jj + 1) * P],
                                          vps[:])

            # ---------- phase 1b: q0 per chunk; h0's first score tiles
            # inline as their q chunks complete ----------
            s00 = score_parts((0, 0))
            for xc in range(NTC):
                cs = slice(xc * TC, (xc + 1) * TC)
                proj_mms(pyps, "acc", lambda e: wq_tile(0, e), xc, 0, ET, hold)
                proj_evac(qA_sb[:, cs], hold, 0)
                rope_chunk(qA_sb, cos_sb, sin_sb, xc)
                s00[xc]()

            # rest of wq; wo prefetch (SP queue: idle once x has issued)
            for ct in range(1, GROUP):
                nc.sync.dma_start(wq_sb[:, ct * ET * P:(ct + 1) * ET * P],
                                  dram["wq"][:, ct * ET * P:(ct + 1) * ET * P])
            for ct in range(GROUP):
                nc.sync.dma_start(wo_sb[:, ct * TT * P:(ct + 1) * TT * P],
                                  dram["wo"][:, ct * TT * P:(ct + 1) * TT * P])
            nc.sync.dma_start(bo16_sb[:], dram["bo16"][:])

            # ---------- phase 2: attention per head ----------
            # scores^T s[tk,tq] per k-tile j with ragged causal width
            # (tq >= 128j); exp on scalar; the exp tiles accumulate
            # elementwise on DVE into egacc, and the softmax denominators
            # come from 4 colsum matmuls over egacc at head end (instead of
            # one ones-matmul per (j, qc)).  Scores of k-tile j+1 are
            # emitted before consuming j so the PE never waits on exp.
            # Projection of head h+1 is interleaved at odd j.
            # out-projection group: one (jt, chunk) PSUM accumulation over
            # the 4 head-column tiles, evacuated with the bias fused
            def ogroup(jt, c, ost, pool):
                op = pool.tile([P, TC], F32,
                               tag="acc" if pool is pyps else "mi",
                               name="ops")
                for ct in range(GROUP):
                    nc.tensor.matmul(
                        op[:],
                        wo_sb[:, (ct * TT + jt) * P:(ct * TT + jt + 1) * P],
                        y_sb[:, ct * T + c * TC:ct * T + (c + 1) * TC],
                        start=(ct == 0), stop=(ct == GROUP - 1))
                dst = ost[:, (c % 2) * TC:(c % 2 + 1) * TC]
                if c % 2 == 0:
                    nc.scalar.activation(
                        dst, op[:], mybir.ActivationFunctionType.Identity,
                        bias=bo16_sb[:, jt:jt + 1])
                else:
                    nc.vector.tensor_scalar_add(dst, op[:],
                                                bo16_sb[:, jt:jt + 1])

            # out-proj chunks 0/1 hide inside h3's attention (which is
            # otherwise exp-bound on Act): (jt, c0) groups start once y
            # chunk 0 is normalized (half0 j>=4), (jt, c1) through half1
            passA_sched = {}
            # c1 (ost-releasing) entries precede c0 (ost-allocating) ones
            # at each step, so the osb/psum slot waits cannot form a cycle
            for i in range(TT):
                passA_sched.setdefault(8 + i // 3, []).append((i, 1))
            for i in range(TT):
                passA_sched.setdefault(4 + i // 2, []).append((i, 0))
            # h3's last iterations are attention-thin: pull the first half
            # of pass B's c2 groups in there too
            for i in range(8):
                passA_sched.setdefault(12 + i // 2, []).append((i, 2))
            osts = {}

            def opass(jt, c):
                if jt not in osts:
                    osts[jt] = osb.tile([P, 2 * TC], BF16, tag="ost",
                                        name="ost")
                ogroup(jt, c, osts[jt], pyps)
                if c == 1:
                    eng = nc.sync if jt % 2 == 0 else nc.scalar
                    eng.dma_start(outt[jt][:, 0:2 * TC], osts.pop(jt)[:])
                elif c == 3:
                    eng = nc.sync if jt % 2 == 0 else nc.scalar
                    eng.dma_start(outt[jt][:, 2 * TC:4 * TC],
                                  osts.pop(jt)[:])

            for h in range(GROUP):
                qN = qB_sb if h % 2 == 0 else qA_sb
                last_head = h == GROUP - 1
                yps = [None] * NTC
                egacc = [None]
                phold = [None]
                pxc = [0]
                def fin_chunk(qc, h=h, egacc=egacc):
                    # chunk qc's denominator: gpsimd all-reduce across
                    # partitions gives the colsum already broadcast, so the
                    # whole normalize chain (reduce + recip + mul) stays off
                    # the PE and off PSUM
                    den_bc = nsb.tile([P, TC], BF16, tag="dn", name="den_bc")
                    nc.gpsimd.partition_all_reduce(
                        den_bc[:], egacc[0][:, qc * TC:(qc + 1) * TC]
                        .bitcast(F32), P, bass_isa.ReduceOp.add)
                    rec_bc = nsb.tile([P, TC], BF16, tag="rc", name="rec_bc")
                    nc.vector.reciprocal(rec_bc[:], den_bc[:])
                    ys = y_sb[:, h * T + qc * TC:h * T + (qc + 1) * TC]
                    nc.vector.tensor_mul(ys, ys, rec_bc[:])

                def next_proj():
                    # emit the next 1/8th of head h+1's projection; rope each
                    # q chunk right after its evacuation so the next head's
                    # scores have nothing left to wait for at the boundary
                    if h >= GROUP - 1 or pxc[0] >= 8:
                        return
                    xc, half = divmod(pxc[0], 2)
                    proj_mms(pmx, "mi", lambda e: wq_tile(h + 1, e), xc,
                             half * 8, (half + 1) * 8, phold)
                    if half == 1:
                        proj_evac(qN[:, xc * TC:(xc + 1) * TC], phold,
                                  h + 1)
                        rope_chunk(qN, cos_sb, sin_sb, xc)
                    pxc[0] += 1

                for j in range(TT):
                    key = (h, j)
                    base = j * P
                    w = T - base
                    eg = egs.pop(key)
                    # mask the diagonal 128-block
                    nc.vector.tensor_mul(eg[:, 0:P], eg[:, 0:P], maskd_sb[:])
                    # denominator accumulation on DVE (elementwise over j)
                    if j == 0:
                        egacc[0] = eap.tile([P, T], F32, tag="ea",
                                            name="egacc")
                        nc.vector.tensor_copy(egacc[0][:], eg[:, 0:T])
                    else:
                        nc.vector.tensor_add(egacc[0][:, base:T],
                                             egacc[0][:, base:T],
                                             eg[:, 0:w])
                    # AV per overlapping q chunk (vtok_j stationary shared)
                    cons = []
                    for qc in range(j // GROUP, NTC):
                        s0 = max(qc * TC, base)
                        s1 = qc * TC + TC

                        def av(qc=qc, s0=s0, s1=s1, j=j, eg=eg, base=base):
                            if j == 0:
                                yps[qc] = pyps.tile([P, TC], F32, tag="acc",
                                                    name="yps")
                            nc.tensor.matmul(
                                yps[qc][:, s0 - qc * TC:s1 - qc * TC],
                                vtok_sb[:, j * P:(j + 1) * P],
                                eg[:, s0 - base:s1 - base],
                                start=(j == 0),
                                stop=(j == GROUP * qc + GROUP - 1))
                            if j == GROUP * qc + GROUP - 1:
                                # evacuate unnormalized: frees the bank
                                nc.scalar.copy(
                                    y_sb[:, h * T + qc * TC:
                                         h * T + (qc + 1) * TC],
                                    yps[qc][:])
                        cons.append(av)
                    # the first segment is the only one that overlaps the
                    # masked diagonal block: consume it last so the mask
                    # (DVE) lands while the other AV segments run
                    cons = cons[1:] + cons[:1]
                    # interleave: the next tiles' score chunks spread
                    # between AV consumption ops, up to two tiles deep so
                    # short late-j iterations never wait on exp
                    sth = []
                    nxt = NEXT.get(key)
                    if nxt is not None and nxt not in egs:
                        sth += score_parts(nxt)
                    nxt2 = NEXT.get(nxt) if nxt is not None else None
                    if nxt2 is not None and nxt2 not in egs:
                        sth += score_parts(nxt2)
                    if j == TT - 4 and h < GROUP - 1:
                        # pull the next head's first (widest) tiles three
                        # steps early: their exps run in this head's late-j
                        # Act slack instead of jamming the next head's start
                        sth += score_parts((h + 1, 0))
                    elif j == TT - 3 and h < GROUP - 1:
                        sth += score_parts((h + 1, 1))
                    ci = si = 0
                    while ci < len(cons) or si < len(sth):
                        if si < len(sth):
                            sth[si]()
                            si += 1
                        for _ in range(2):
                            if ci < len(cons):
                                cons[ci]()
                                ci += 1
                    if j % GROUP == GROUP - 1:
                        # chunk j//4's denominator is final: normalize its
                        # y in place (reduce + recip + mul, all off-PE)
                        fin_chunk(j // GROUP)
                    next_proj()
                    if last_head:
                        for jt, c in passA_sched.get(j, []):
                            opass(jt, c)



            # ---------- phase 3: out-projection chunks 2 and 3 ----------
            # the last j-tile ships per-chunk on both queues so the final
            # evac+DMA tail is as short as possible
            for jt in range(TT):
                last = jt == TT - 1
                if jt < 8:
                    # c2 already ran inside h3's tail; just finish c3
                    ost = osts.pop(jt)
                else:
                    ost = osb.tile([P, 2 * TC], BF16, tag="ost", name="ost")
                    ogroup(jt, 2, ost, pyps)
                if last:
                    nc.sync.dma_start(outt[jt][:, 2 * TC:3 * TC],
                                      ost[:, 0:TC])
                    # final group in halves with immediate half DMAs so the
                    # closing evac+DMA chain is as short as possible
                    for hh2 in range(2):
                        op = pmx.tile([P, TC], F32, tag="mi", name="ops")
                        sl = slice(hh2 * TC // 2, (hh2 + 1) * TC // 2)
                        for ct in range(GROUP):
                            nc.tensor.matmul(
                                op[:, 0:TC // 2],
                                wo_sb[:, (ct * TT + jt) * P:
                                      (ct * TT + jt + 1) * P],
                                y_sb[:, ct * T + 3 * TC + hh2 * TC // 2:
                                     ct * T + 3 * TC + (hh2 + 1) * TC // 2],
                                start=(ct == 0), stop=(ct == GROUP - 1))
                        dst = ost[:, TC + hh2 * TC // 2:
                                  TC + (hh2 + 1) * TC // 2]
                        if hh2 == 0:
                            nc.scalar.activation(
                                dst, op[:, 0:TC // 2],
                                mybir.ActivationFunctionType.Identity,
                                bias=bo16_sb[:, jt:jt + 1])
                            nc.scalar.dma_start(
                                outt[jt][:, 3 * TC:3 * TC + TC // 2], dst)
                        else:
                            nc.vector.tensor_scalar_add(
                                dst, op[:, 0:TC // 2], bo16_sb[:, jt:jt + 1])
                            nc.sync.dma_start(
                                outt[jt][:, 3 * TC + TC // 2:4 * TC], dst)
                else:
                    ogroup(jt, 3, ost, pmx)
                    eng = nc.sync if jt % 2 == 0 else nc.scalar
                    eng.dma_start(outt[jt][:, 2 * TC:4 * TC], ost[:])

    nc.compile()
    _CACHE["nc"] = nc
    return nc


def _host_inputs(x, Wq, bq, Wk, bk, Wv, bv, Wo, bo):
    """Per-core input dicts (bf16 layouts matching the DRAM decls)."""
    f = np.float32
    i = np.arange(1, D // 2 + 1, dtype=np.float64)
    thetas = ROPE_BASE ** (-2.0 * (i - 1.0) / D)
    ang = np.arange(1, T + 1, dtype=np.float64)[:, None] * thetas      # [T, D/2]
    cos = np.concatenate([np.cos(ang), np.cos(ang)], axis=1).T
    sin = np.concatenate([np.sin(ang), np.sin(ang)], axis=1).T
    # sign-folded sin for the partition-offset rope: rows<64 negated
    sin_s = sin.copy()
    sin_s[: D // 2] *= -1.0
    cosb = np.ascontiguousarray(cos.astype(bfloat16))
    sinb = np.ascontiguousarray(sin_s.astype(bfloat16))
    # fold the 1/sqrt(D) score scale into the q projection so q and k share
    # one pair of rope tables
    s = np.float32(1.0 / np.sqrt(D))
    Wq = Wq * s
    bq = bq * s

    ident = np.eye(P, dtype=bfloat16)
    pcol = np.arange(P)[:, None]
    fcol = np.arange(P)[None, :]
    maskd = np.ascontiguousarray((pcol <= fcol).astype(bfloat16))

    # xT per batch, chunked contiguous: (NTC, ET, P, TC)
    xts = []
    for b in range(B):
        xb = x[b].astype(bfloat16)                                     # [T, E]
        xt = np.ascontiguousarray(
            xb.T.reshape(ET, P, NTC, TC).transpose(2, 1, 0, 3))
        xts.append(xt)

    per_core = []
    for c in range(N_CORES):
        b, h = divmod(c, GROUP)
        WqS = Wq[h * KV:(h + 1) * KV, :]                               # [512, E]
        wq = np.ascontiguousarray(
            WqS.T.reshape(ET, P, GROUP, P).transpose(1, 2, 0, 3)
            .reshape(P, -1).astype(bfloat16))
        WkS = Wk[h * D:(h + 1) * D, :]
        wk = np.ascontiguousarray(
            WkS.T.reshape(ET, P, P).transpose(1, 0, 2).reshape(P, -1)
            .astype(bfloat16))
        WvS = Wv[h * D:(h + 1) * D, :]
        wv = np.ascontiguousarray(
            WvS.T.reshape(ET, P, P).transpose(1, 0, 2).reshape(P, -1)
            .astype(bfloat16))
        WoS = Wo[:, h * KV:(h + 1) * KV]                               # [E, 512]
        wo = np.ascontiguousarray(
            WoS.T.reshape(GROUP, P, TT, P).transpose(1, 0, 2, 3).reshape(P, -1)
            .astype(bfloat16))
        bias6 = np.stack([bq[h * KV + ct * P: h * KV + (ct + 1) * P]
                          for ct in range(GROUP)]
                         + [bk[h * D:(h + 1) * D], bv[h * D:(h + 1) * D]],
                         axis=1).astype(f)
        bo16 = (bo.reshape(TT, P).T if h == 0
                else np.zeros((P, TT), f)).astype(f)
        per_core.append({
            "xt": xts[b], "wq": wq, "wk": wk, "wv": wv, "wo": wo,
            "bias6": np.ascontiguousarray(bias6),
            "bo16": np.ascontiguousarray(bo16),
            "cos": cosb, "sin": sinb,
            "ident": ident, "maskd": maskd,
        })
    return per_core


def kernel(**inputs):
    x = np.asarray(inputs["x"], np.float32)
    nc = _build_program()
    in_maps = _host_inputs(
        x, *(np.asarray(inputs[k], np.float32)
             for k in ("Wq", "bq", "Wk", "bk", "Wv", "bv", "Wo", "bo")))
    res = run_bass_kernel_spmd(nc, in_maps, list(range(N_CORES)))
    out = np.empty((B, T, E), np.float32)
    for b in range(B):
        acc = np.zeros((E, T), np.float32)
        for h in range(GROUP):
            acc += res.results[b * GROUP + h]["outt"].reshape(E, T)
        out[b] = acc.T
    return out
